# revision 11
# baseline (speedup 1.0000x reference)
# MultiHeadCrossAttention Trainium2 Bass/Tile kernel.
#
# Problem: B=8, NQ=1024, NK=2048, EMB=1024, H=16, D=64 (fp32 I/O).
#   q = query_tokens @ Wq + bq ; k = image_embeds @ Wk + bk ; v = image_embeds @ Wv + bv
#   att = softmax(q k^T / sqrt(EMB)) ; out = (att v) @ Wp + bp
#
# Sharding: data-parallel over batch — core b computes batch element b. No collectives.
#
# Per-core dataflow (all layouts chosen so TensorE contraction is always on partitions):
#   xqT/xkT  = transposed inputs  [emb_in(part-tiles), tokens]   (PE transpose, fp32->fp16)
#   qT,kT    = Wq/Wk proj outputs [emb(part), tokens] fp16       (bk dropped: softmax-invariant)
#   vones    = V proj [tok(part), head, 64+1] fp16, col 64 = 1.0 (ones col makes PV also
#              produce the softmax denominator row; bv folded in after normalization)
#   eT       = K_h^T.T @ Q_h^T -> PSUM [k-tok(part), q]          (per head, per 128-tok tile)
#   expT     = exp(eT/32) fp16 (ScalarE, scale fused; logits are ~N(0,0.083) so no
#              max-subtraction is needed for stability)
#   O_unnorm = vones.T @ expT -> PSUM [65, q]: rows 0-63 = head out^T, row 64 = sum_k exp
#   att_outT = O/S gathered to [emb(part), q] fp16, normalized via 1/S broadcast from a
#              tiny PE matmul (E-matrix), + bv
#   y        = att_outT.T @ Wp + (ones outer bp) -> [q(part), emb] fp32 -> DRAM
import numpy as np

import concourse.bass as bass
import concourse.mybir as mybir
import concourse.tile as tile
from concourse import bacc
from concourse.masks import make_identity

F32 = mybir.dt.float32
F16 = mybir.dt.float16

B, NQ, NK = 8, 1024, 2048
EMB = 1024
H = 16
D = 64
P = 128
NCORES = 8

QT_TILES = NQ // P        # 8 q-token tiles
KT_TILES = NK // P        # 16 k-token tiles
EB = EMB // P             # 8 emb blocks
SCALE = 1.0 / float(np.sqrt(EMB))


def build_ir(nc, debug_taps=False):
    xq = nc.dram_tensor("query_tokens", [NQ, EMB], F32, kind="ExternalInput")
    xkv = nc.dram_tensor("image_embeds", [NK, EMB], F32, kind="ExternalInput")
    wq_d = nc.dram_tensor("Wq", [EMB, EMB], F32, kind="ExternalInput")
    wk_d = nc.dram_tensor("Wk", [EMB, EMB], F32, kind="ExternalInput")
    wv_d = nc.dram_tensor("Wv", [EMB, EMB], F32, kind="ExternalInput")
    wp_d = nc.dram_tensor("Wp", [EMB, EMB], F32, kind="ExternalInput")
    bq_d = nc.dram_tensor("bq", [EMB], F32, kind="ExternalInput")
    bv_d = nc.dram_tensor("bv", [EMB], F32, kind="ExternalInput")
    bp_d = nc.dram_tensor("bp", [EMB], F32, kind="ExternalInput")
    y = nc.dram_tensor("y", [NQ, EMB], F32, kind="ExternalOutput")
    dbg = {}
    if debug_taps:
        dbg["qT"] = nc.dram_tensor("dbg_qT", [P, EB, NQ], F16, kind="ExternalOutput")
        dbg["kT"] = nc.dram_tensor("dbg_kT", [P, EB, NK], F16, kind="ExternalOutput")
        dbg["vones"] = nc.dram_tensor(
            "dbg_vones", [P, KT_TILES, H, D + 1], F16, kind="ExternalOutput"
        )
        dbg["emat"] = nc.dram_tensor("dbg_emat", [H, EB, P], F16, kind="ExternalOutput")
        dbg["s_all"] = nc.dram_tensor("dbg_s_all", [H, NQ], F32, kind="ExternalOutput")
        dbg["att"] = nc.dram_tensor("dbg_att", [P, EB, NQ], F16, kind="ExternalOutput")
        dbg["xqT"] = nc.dram_tensor("dbg_xqT", [P, EB, NQ], F16, kind="ExternalOutput")
        dbg["bpp"] = nc.dram_tensor("dbg_bpp", [1, EMB], F16, kind="ExternalOutput")

    with tile.TileContext(nc) as tc:
        with tc.tile_pool(name="persist", bufs=1) as pp:
            ident = pp.tile([P, P], F32, tag="ident")
            make_identity(nc, ident)
            bq_sb = pp.tile([P, EB], F32, tag="bq")
            bv_sb = pp.tile([P, EB], F32, tag="bv")
            with nc.allow_non_contiguous_dma(reason="tiny bias loads"):
                nc.sync.dma_start(bq_sb, bq_d[:].rearrange("(b p) -> p b", p=P))
                nc.sync.dma_start(bv_sb, bv_d[:].rearrange("(b p) -> p b", p=P))
            ones_row = pp.tile([1, P], F16, tag="ones_row")
            nc.vector.memset(ones_row, 1.0)
            bv16 = pp.tile([P, EB], F16, tag="bv16")
            nc.vector.tensor_copy(out=bv16, in_=bv_sb)

            qT = pp.tile([P, EB, NQ], F16, tag="qT")
            kT = pp.tile([P, EB, NK], F16, tag="kT")
            vones = pp.tile([P, KT_TILES, H, D + 1], F16, tag="vones")
            nc.vector.memset(vones[:, :, :, D : D + 1], 1.0)
            wp = pp.tile([P, EB, EMB], F16, tag="wp")
            bpp = pp.tile([1, EMB], F16, tag="bpp")  # bv @ Wp + bp

            # ---------------- phase A1+B: transpose x_q, project Q, prep Wp/bpp -------
            with (
                tc.tile_pool(name="phB", bufs=1) as pb,
                tc.tile_pool(name="psumAB", bufs=1, space="PSUM") as psAB,
            ):
                xqT = pb.tile([P, EB, NQ], F16, tag="xqT")
                wq = pb.tile([P, EB, EMB], F16, tag="wq")
                bp_sb = pb.tile([1, EMB], F32, tag="bp_sb")
                nc.sync.dma_start(bp_sb, bp_d[None, :])

                # load + cast Wq and Wp (Wp needed for bpp and phase D)
                for w16, wd in ((wq, wq_d), (wp, wp_d)):
                    for kb in range(EB):
                        wstage = pb.tile([P, EMB], F32, tag="wstage", bufs=2)
                        nc.sync.dma_start(wstage, wd[kb * P : (kb + 1) * P, :])
                        nc.any.tensor_copy(out=w16[:, kb, :], in_=wstage)

                # transpose x_q: [tok, emb] -> xqT [emb(part), tok]
                for tt in range(QT_TILES):
                    xnat = pb.tile([P, EMB], F32, tag="xnat", bufs=2)
                    nc.sync.dma_start(xnat, xq[tt * P : (tt + 1) * P, :])
                    for g in range(2):
                        tp = psAB.tile([P, 4 * P], F32, tag="tp", bufs=2)
                        for e4 in range(4):
                            eb = 4 * g + e4
                            nc.tensor.transpose(
                                tp[:, e4 * P : (e4 + 1) * P],
                                xnat[:, eb * P : (eb + 1) * P],
                                ident,
                            )
                        nc.any.tensor_copy(
                            out=xqT[:, 4 * g : 4 * g + 4, tt * P : (tt + 1) * P],
                            in_=tp.rearrange("p (b f) -> p b f", b=4),
                        )

                # Q projection: qT[emb, q] = Wq.T-contraction, + bq, cast fp16
                for mo in range(EB):
                    for nb in range(NQ // 512):
                        psq = psAB.tile([P, 512], F32, tag="pj", bufs=3)
                        for kb in range(EB):
                            nc.tensor.matmul(
                                psq,
                                lhsT=wq[:, kb, mo * P : (mo + 1) * P],
                                rhs=xqT[:, kb, nb * 512 : (nb + 1) * 512],
                                start=(kb == 0),
                                stop=(kb == EB - 1),
                            )
                        nc.any.tensor_scalar_add(
                            qT[:, mo, nb * 512 : (nb + 1) * 512],
                            psq,
                            bq_sb[:, mo : mo + 1],
                        )

                # bpp = bv @ Wp + bp  (rank-1 bias prep for the output projection)
                for nb in range(EMB // 512):
                    psb = psAB.tile([1, 512], F32, tag="bp_ps", bufs=1)
                    for kb in range(EB):
                        nc.tensor.matmul(
                            psb,
                            lhsT=bv16[:, kb : kb + 1],
                            rhs=wp[:, kb, nb * 512 : (nb + 1) * 512],
                            start=(kb == 0),
                            stop=(kb == EB - 1),
                        )
                    nc.vector.tensor_tensor(
                        bpp[0:1, nb * 512 : (nb + 1) * 512],
                        psb,
                        bp_sb[0:1, nb * 512 : (nb + 1) * 512],
                        mybir.AluOpType.add,
                    )

                if debug_taps:
                    nc.sync.dma_start(dbg["xqT"][:], xqT[:])
                    nc.sync.dma_start(dbg["qT"][:], qT[:])
                    nc.sync.dma_start(dbg["bpp"][:], bpp[:])

            # ---------------- phase A2+C: transpose x_kv, project K and V ------------
            with (
                tc.tile_pool(name="phC", bufs=1) as pc,
                tc.tile_pool(name="psumC", bufs=1, space="PSUM") as psC,
            ):
                xkT = pc.tile([P, EB, NK], F16, tag="xkT")
                wk = pc.tile([P, EB, EMB], F16, tag="wk")
                wv = pc.tile([P, EB, EMB], F16, tag="wv")
                for w16, wd in ((wk, wk_d), (wv, wv_d)):
                    for kb in range(EB):
                        wstage = pc.tile([P, EMB], F32, tag="wstage", bufs=2)
                        nc.sync.dma_start(wstage, wd[kb * P : (kb + 1) * P, :])
                        nc.any.tensor_copy(out=w16[:, kb, :], in_=wstage)

                for tt in range(KT_TILES):
                    xnat = pc.tile([P, EMB], F32, tag="xnat", bufs=2)
                    nc.sync.dma_start(xnat, xkv[tt * P : (tt + 1) * P, :])
                    for g in range(2):
                        tp = psC.tile([P, 4 * P], F32, tag="tp", bufs=2)
                        for e4 in range(4):
                            eb = 4 * g + e4
                            nc.tensor.transpose(
                                tp[:, e4 * P : (e4 + 1) * P],
                                xnat[:, eb * P : (eb + 1) * P],
                                ident,
                            )
                        nc.any.tensor_copy(
                            out=xkT[:, 4 * g : 4 * g + 4, tt * P : (tt + 1) * P],
                            in_=tp.rearrange("p (b f) -> p b f", b=4),
                        )

                # K projection (no bias: bk is softmax-invariant)
                for mo in range(EB):
                    for nb in range(NK // 512):
                        psk = psC.tile([P, 512], F32, tag="pj", bufs=3)
                        for kb in range(EB):
                            nc.tensor.matmul(
                                psk,
                                lhsT=wk[:, kb, mo * P : (mo + 1) * P],
                                rhs=xkT[:, kb, nb * 512 : (nb + 1) * 512],
                                start=(kb == 0),
                                stop=(kb == EB - 1),
                            )
                        nc.any.tensor_copy(
                            out=kT[:, mo, nb * 512 : (nb + 1) * 512], in_=psk
                        )

                # V projection -> vones [tok(part), tok-tile, head, 0:64]  (bv deferred)
                for mt in range(KT_TILES):
                    for nb in range(EMB // 512):
                        psv = psC.tile([P, 512], F32, tag="pj", bufs=3)
                        for kb in range(EB):
                            nc.tensor.matmul(
                                psv,
                                lhsT=xkT[:, kb, mt * P : (mt + 1) * P],
                                rhs=wv[:, kb, nb * 512 : (nb + 1) * 512],
                                start=(kb == 0),
                                stop=(kb == EB - 1),
                            )
                        nc.any.tensor_copy(
                            out=vones[:, mt, 8 * nb : 8 * nb + 8, 0:D],
                            in_=psv.rearrange("p (h d) -> p h d", h=8),
                        )

                if debug_taps:
                    nc.sync.dma_start(dbg["kT"][:], kT[:])
                    nc.sync.dma_start(dbg["vones"][:], vones[:])

            # ---------------- phase D: attention + output projection ------------------
            with (
                tc.tile_pool(name="phD", bufs=1) as pd,
                tc.tile_pool(name="psumE", bufs=1, space="PSUM") as psE,
                tc.tile_pool(name="psumPV", bufs=1, space="PSUM") as psPV,
                tc.tile_pool(name="psumMisc", bufs=1, space="PSUM") as psM,
            ):
                att = pd.tile([P, EB, NQ], F16, tag="att")
                s_all = pd.tile([H, NQ], F32, tag="s_all")
                srec = pd.tile([H, NQ], F16, tag="srec")
                # E[s, t, p] = 1 iff s == 2t + (p >= 64): broadcast selector so a
                # [16, q] vector of per-head values scatters to [128, q] tiles.
                emat = pd.tile([H, EB, P], F16, tag="emat")
                nc.vector.memset(emat, 0.0)
                nc.gpsimd.affine_select(
                    out=emat[:, :, 0:D],
                    in_=emat[:, :, 0:D],
                    # iota = s - 2t over (partition s, free (t, p<64))
                    pattern=[[-2, EB], [0, D]],
                    channel_multiplier=1,
                    base=0,
                    compare_op=mybir.AluOpType.not_equal,
                    fill=1.0,
                )
                nc.gpsimd.affine_select(
                    out=emat[:, :, D:P],
                    in_=emat[:, :, D:P],
                    # iota = s - 2t - 1 over (partition s, free (t, p>=64))
                    pattern=[[-2, EB], [0, D]],
                    channel_multiplier=1,
                    base=-1,
                    compare_op=mybir.AluOpType.not_equal,
                    fill=1.0,
                )

                for qh in range(NQ // 512):
                    qs = slice(qh * 512, (qh + 1) * 512)
                    for hp in range(H // 2):
                        eb = hp
                        pv_ps = [
                            psPV.tile(
                                [D + 1, 512], F32, tag="pv", bufs=2, name=f"pv{s}"
                            )
                            for s in range(2)
                        ]
                        for jh in range(2):
                            ex = pd.tile([P, 2, 8, 512], F16, tag="expT", bufs=3)
                            for j in range(8):
                                kt = jh * 8 + j
                                pe = psE.tile([P, 2, 512], F32, tag="eT", bufs=2)
                                for s in range(2):
                                    r = slice(64 * s, 64 * s + 64)
                                    nc.tensor.matmul(
                                        pe[:, s, :],
                                        lhsT=kT[r, eb, kt * P : (kt + 1) * P],
                                        rhs=qT[r, eb, qs],
                                        start=True,
                                        stop=True,
                                    )
                                nc.scalar.activation(
                                    ex[:, :, j, :],
                                    pe,
                                    mybir.ActivationFunctionType.Exp,
                                    bias=0.0,
                                    scale=SCALE,
                                )
                                for s in range(2):
                                    nc.tensor.matmul(
                                        pv_ps[s],
                                        lhsT=vones[:, kt, 2 * hp + s, :],
                                        rhs=ex[:, s, j, :],
                                        start=(kt == 0),
                                        stop=(kt == KT_TILES - 1),
                                    )
                        # evacuate PV: out rows -> att (head s at partitions 64s..),
                        # denominator row 64 -> s_all via staging + SBUF-to-SBUF DMA
                        for s in range(2):
                            h = 2 * hp + s
                            nc.vector.tensor_copy(
                                out=att[64 * s : 64 * s + 64, eb, qs],
                                in_=pv_ps[s][0:D, :],
                            )
                            sst = pd.tile([65, 512], F32, tag="sstage", bufs=2)
                            nc.vector.tensor_copy(
                                out=sst[64:65, :], in_=pv_ps[s][D : D + 1, :]
                            )
                            nc.sync.dma_start(s_all[h : h + 1, qs], sst[64:65, :])

                    # normalization: srec = 1/s_all ; broadcast with E-matmul; * att; + bv
                    with nc.allow_low_precision(
                        reason="1/S in fp16: one multiplicative factor, ~5e-4 rel"
                    ):
                        nc.vector.reciprocal(srec[:, qs], s_all[:, qs])
                    for t in range(EB):
                        psb = psM.tile([P, 512], F32, tag="srecB", bufs=1)
                        nc.tensor.matmul(
                            psb,
                            lhsT=emat[:, t, :],
                            rhs=srec[:, qs],
                            start=True,
                            stop=True,
                        )
                        # NOTE: bv is NOT added here — softmax rows sum to 1, so
                        # bv's contribution to y is exactly bv @ Wp, folded into bpp.
                        nc.vector.tensor_tensor(
                            att[:, t, qs], att[:, t, qs], psb, mybir.AluOpType.mult
                        )

                    # output projection for this q-half
                    for qt in range(4):
                        rows = slice(qh * 512 + qt * P, qh * 512 + (qt + 1) * P)
                        for nb in range(EMB // 512):
                            py = psM.tile([P, 512], F32, tag="y", bufs=1)
                            for kb in range(EB):
                                nc.tensor.matmul(
                                    py,
                                    lhsT=att[:, kb, rows],
                                    rhs=wp[:, kb, nb * 512 : (nb + 1) * 512],
                                    start=(kb == 0),
                                    stop=False,
                                )
                            nc.tensor.matmul(
                                py,
                                lhsT=ones_row,
                                rhs=bpp[0:1, nb * 512 : (nb + 1) * 512],
                                start=False,
                                stop=True,
                            )
                            ysb = pd.tile([P, 512], F32, tag="ysb", bufs=2)
                            nc.vector.tensor_copy(out=ysb, in_=py)
                            nc.sync.dma_start(
                                y[rows, nb * 512 : (nb + 1) * 512], ysb
                            )
                if debug_taps:
                    nc.sync.dma_start(dbg["emat"][:], emat[:])
                    nc.sync.dma_start(dbg["s_all"][:], s_all[:])
                    nc.sync.dma_start(dbg["att"][:], att[:])
    return nc


_CACHED = None


def build():
    global _CACHED
    if _CACHED is None:
        nc = bacc.Bacc("TRN2", target_bir_lowering=False, debug=False)
        build_ir(nc)
        nc.compile()
        _CACHED = nc
    return _CACHED


INPUT_NAMES = [
    "query_tokens",
    "image_embeds",
    "Wq",
    "Wk",
    "Wv",
    "Wp",
    "bq",
    "bv",
    "bp",
]


def make_in_maps(inputs):
    arrs = {k: np.ascontiguousarray(np.asarray(v, dtype=np.float32)) for k, v in inputs.items()}
    in_maps = []
    for b in range(NCORES):
        m = {
            "query_tokens": np.ascontiguousarray(arrs["query_tokens"][b]),
            "image_embeds": np.ascontiguousarray(arrs["image_embeds"][b]),
        }
        for n in ("Wq", "Wk", "Wv", "Wp", "bq", "bv", "bp"):
            m[n] = arrs[n]
        in_maps.append(m)
    return in_maps


def run(inputs, trace=False, **kwargs):
    from concourse.bass_utils import run_bass_kernel_spmd

    nc = build()
    res = run_bass_kernel_spmd(
        nc, make_in_maps(inputs), core_ids=list(range(NCORES)), trace=trace, **kwargs
    )
    out = np.stack([r["y"] for r in res.results], axis=0)
    return out, res


def kernel(**inputs) -> np.ndarray:
    out, _ = run(inputs, trace=False)
    return out


# revision 16
# speedup vs baseline: 1.0248x; 1.0248x over previous
# MultiHeadCrossAttention Trainium2 Bass/Tile kernel.
#
# Problem: B=8, NQ=1024, NK=2048, EMB=1024, H=16, D=64 (fp32 I/O).
#   q = query_tokens @ Wq + bq ; k = image_embeds @ Wk + bk ; v = image_embeds @ Wv + bv
#   att = softmax(q k^T / sqrt(EMB)) ; out = (att v) @ Wp + bp
#
# Sharding: data-parallel over batch — core b computes batch element b. No collectives.
#
# Per-core dataflow (all layouts chosen so TensorE contraction is always on partitions):
#   xqT/xkT  = transposed inputs  [emb_in(part-tiles), tokens]   (PE transpose, fp32->fp16)
#   qT,kT    = Wq/Wk proj outputs [emb(part), tokens] fp16       (bk dropped: softmax-invariant)
#   vones    = V proj [tok(part), head, 64+1] fp16, col 64 = 1.0 (ones col makes PV also
#              produce the softmax denominator row; bv folded in after normalization)
#   eT       = K_h^T.T @ Q_h^T -> PSUM [k-tok(part), q]          (per head, per 128-tok tile)
#   expT     = exp(eT/32) fp16 (ScalarE, scale fused; logits are ~N(0,0.083) so no
#              max-subtraction is needed for stability)
#   O_unnorm = vones.T @ expT -> PSUM [65, q]: rows 0-63 = head out^T, row 64 = sum_k exp
#   att_outT = O/S gathered to [emb(part), q] fp16, normalized via 1/S broadcast from a
#              tiny PE matmul (E-matrix), + bv
#   y        = att_outT.T @ Wp + (ones outer bp) -> [q(part), emb] fp32 -> DRAM
import numpy as np

import concourse.bass as bass
import concourse.mybir as mybir
import concourse.tile as tile
from concourse import bacc
from concourse.masks import make_identity

F32 = mybir.dt.float32
F16 = mybir.dt.float16

B, NQ, NK = 8, 1024, 2048
EMB = 1024
H = 16
D = 64
P = 128
NCORES = 8

QT_TILES = NQ // P        # 8 q-token tiles
KT_TILES = NK // P        # 16 k-token tiles
EB = EMB // P             # 8 emb blocks
SCALE = 1.0 / float(np.sqrt(EMB))


def build_ir(nc, debug_taps=False):
    xq = nc.dram_tensor("query_tokens", [NQ, EMB], F32, kind="ExternalInput")
    xkv = nc.dram_tensor("image_embeds", [NK, EMB], F32, kind="ExternalInput")
    wq_d = nc.dram_tensor("Wq", [EMB, EMB], F32, kind="ExternalInput")
    wk_d = nc.dram_tensor("Wk", [EMB, EMB], F32, kind="ExternalInput")
    wv_d = nc.dram_tensor("Wv", [EMB, EMB], F32, kind="ExternalInput")
    wp_d = nc.dram_tensor("Wp", [EMB, EMB], F32, kind="ExternalInput")
    bq_d = nc.dram_tensor("bq", [EMB], F32, kind="ExternalInput")
    bv_d = nc.dram_tensor("bv", [EMB], F32, kind="ExternalInput")
    bp_d = nc.dram_tensor("bp", [EMB], F32, kind="ExternalInput")
    y = nc.dram_tensor("y", [NQ, EMB], F32, kind="ExternalOutput")
    dbg = {}
    if debug_taps:
        dbg["qT"] = nc.dram_tensor("dbg_qT", [P, EB, NQ], F16, kind="ExternalOutput")
        dbg["kT"] = nc.dram_tensor("dbg_kT", [P, EB, NK], F16, kind="ExternalOutput")
        dbg["vones"] = nc.dram_tensor(
            "dbg_vones", [P, KT_TILES, H, D + 1], F16, kind="ExternalOutput"
        )
        dbg["emat"] = nc.dram_tensor("dbg_emat", [H, EB, P], F16, kind="ExternalOutput")
        dbg["s_all"] = nc.dram_tensor("dbg_s_all", [H, NQ], F32, kind="ExternalOutput")
        dbg["att"] = nc.dram_tensor("dbg_att", [P, EB, NQ], F16, kind="ExternalOutput")
        dbg["xqT"] = nc.dram_tensor("dbg_xqT", [P, EB, NQ], F16, kind="ExternalOutput")
        dbg["bpp"] = nc.dram_tensor("dbg_bpp", [1, EMB], F16, kind="ExternalOutput")

    with tile.TileContext(nc) as tc:
        with tc.tile_pool(name="persist", bufs=1) as pp:
            ident = pp.tile([P, P], F16, tag="ident")
            make_identity(nc, ident)
            bq_sb = pp.tile([P, EB], F32, tag="bq")
            bv_sb = pp.tile([P, EB], F32, tag="bv")
            with nc.allow_non_contiguous_dma(reason="tiny bias loads"):
                nc.sync.dma_start(bq_sb, bq_d[:].rearrange("(b p) -> p b", p=P))
                nc.sync.dma_start(bv_sb, bv_d[:].rearrange("(b p) -> p b", p=P))
            ones_row = pp.tile([1, P], F16, tag="ones_row")
            nc.vector.memset(ones_row, 1.0)
            bv16 = pp.tile([P, EB], F16, tag="bv16")
            nc.vector.tensor_copy(out=bv16, in_=bv_sb)

            qT = pp.tile([P, EB, NQ], F16, tag="qT")
            kT = pp.tile([P, EB, NK], F16, tag="kT")
            vones = pp.tile([P, KT_TILES, H, D + 1], F16, tag="vones")
            nc.vector.memset(vones[:, :, :, D : D + 1], 1.0)
            wp = pp.tile([P, EB, EMB], F16, tag="wp")
            bpp = pp.tile([1, EMB], F16, tag="bpp")  # bv @ Wp + bp

            # ---------------- phase A1+B: transpose x_q, project Q, prep Wp/bpp -------
            with (
                tc.tile_pool(name="phB", bufs=1) as pb,
                tc.tile_pool(name="psumAB", bufs=1, space="PSUM") as psAB,
            ):
                xqT = pb.tile([P, EB, NQ], F16, tag="xqT")
                wq = pb.tile([P, EB, EMB], F16, tag="wq")
                bp_sb = pb.tile([1, EMB], F32, tag="bp_sb")
                nc.sync.dma_start(bp_sb, bp_d[None, :])

                # load + cast Wq and Wp (Wp needed for bpp and phase D)
                for w16, wd in ((wq, wq_d), (wp, wp_d)):
                    for kb in range(EB):
                        wstage = pb.tile([P, EMB], F32, tag="wstage", bufs=2)
                        nc.sync.dma_start(wstage, wd[kb * P : (kb + 1) * P, :])
                        nc.any.tensor_copy(out=w16[:, kb, :], in_=wstage)

                # transpose x_q: [tok, emb] -> xqT [emb(part), tok]; cast to fp16
                # first so the PE transpose runs at 1 cyc/row instead of 2.
                for tt in range(QT_TILES):
                    xnat = pb.tile([P, EMB], F32, tag="xnat", bufs=3)
                    nc.sync.dma_start(xnat, xq[tt * P : (tt + 1) * P, :])
                    xnat16 = pb.tile([P, EMB], F16, tag="xnat16", bufs=3)
                    nc.vector.tensor_copy(out=xnat16, in_=xnat)
                    for g in range(2):
                        tp = psAB.tile([P, 4 * P], F16, tag="tp", bufs=2)
                        for e4 in range(4):
                            eb = 4 * g + e4
                            nc.tensor.transpose(
                                tp[:, e4 * P : (e4 + 1) * P],
                                xnat16[:, eb * P : (eb + 1) * P],
                                ident,
                            )
                        nc.any.tensor_copy(
                            out=xqT[:, 4 * g : 4 * g + 4, tt * P : (tt + 1) * P],
                            in_=tp.rearrange("p (b f) -> p b f", b=4),
                        )

                # Q projection: qT[emb, q] = Wq.T-contraction, + bq, cast fp16.
                # nb-inner so consecutive matmuls reuse the loaded weights.
                for mo in range(EB):
                    psq = [
                        psAB.tile([P, 512], F32, tag="pj", bufs=4, name=f"psq{nb}")
                        for nb in range(NQ // 512)
                    ]
                    for kb in range(EB):
                        for nb in range(NQ // 512):
                            nc.tensor.matmul(
                                psq[nb],
                                lhsT=wq[:, kb, mo * P : (mo + 1) * P],
                                rhs=xqT[:, kb, nb * 512 : (nb + 1) * 512],
                                start=(kb == 0),
                                stop=(kb == EB - 1),
                            )
                    for nb in range(NQ // 512):
                        nc.any.tensor_scalar_add(
                            qT[:, mo, nb * 512 : (nb + 1) * 512],
                            psq[nb],
                            bq_sb[:, mo : mo + 1],
                        )

                # bpp = bv @ Wp + bp  (rank-1 bias prep for the output projection)
                for nb in range(EMB // 512):
                    psb = psAB.tile([1, 512], F32, tag="bp_ps", bufs=1)
                    for kb in range(EB):
                        nc.tensor.matmul(
                            psb,
                            lhsT=bv16[:, kb : kb + 1],
                            rhs=wp[:, kb, nb * 512 : (nb + 1) * 512],
                            start=(kb == 0),
                            stop=(kb == EB - 1),
                        )
                    nc.vector.tensor_tensor(
                        bpp[0:1, nb * 512 : (nb + 1) * 512],
                        psb,
                        bp_sb[0:1, nb * 512 : (nb + 1) * 512],
                        mybir.AluOpType.add,
                    )

                if debug_taps:
                    nc.sync.dma_start(dbg["xqT"][:], xqT[:])
                    nc.sync.dma_start(dbg["qT"][:], qT[:])
                    nc.sync.dma_start(dbg["bpp"][:], bpp[:])

            # ---------------- phase A2+C: transpose x_kv, project K and V ------------
            with (
                tc.tile_pool(name="phC", bufs=1) as pc,
                tc.tile_pool(name="psumC", bufs=1, space="PSUM") as psC,
            ):
                xkT = pc.tile([P, EB, NK], F16, tag="xkT")
                wk = pc.tile([P, EB, EMB], F16, tag="wk")
                wv = pc.tile([P, EB, EMB], F16, tag="wv")
                for w16, wd in ((wk, wk_d), (wv, wv_d)):
                    for kb in range(EB):
                        wstage = pc.tile([P, EMB], F32, tag="wstage", bufs=2)
                        nc.sync.dma_start(wstage, wd[kb * P : (kb + 1) * P, :])
                        nc.any.tensor_copy(out=w16[:, kb, :], in_=wstage)

                for tt in range(KT_TILES):
                    xnat = pc.tile([P, EMB], F32, tag="xnat", bufs=3)
                    nc.sync.dma_start(xnat, xkv[tt * P : (tt + 1) * P, :])
                    xnat16 = pc.tile([P, EMB], F16, tag="xnat16", bufs=3)
                    nc.vector.tensor_copy(out=xnat16, in_=xnat)
                    for g in range(2):
                        tp = psC.tile([P, 4 * P], F16, tag="tp", bufs=2)
                        for e4 in range(4):
                            eb = 4 * g + e4
                            nc.tensor.transpose(
                                tp[:, e4 * P : (e4 + 1) * P],
                                xnat16[:, eb * P : (eb + 1) * P],
                                ident,
                            )
                        nc.any.tensor_copy(
                            out=xkT[:, 4 * g : 4 * g + 4, tt * P : (tt + 1) * P],
                            in_=tp.rearrange("p (b f) -> p b f", b=4),
                        )

                # K projection (no bias: bk is softmax-invariant); nb-inner pairs
                # reuse loaded weights
                for mo in range(EB):
                    for nh in range(NK // 1024):
                        psk = [
                            psC.tile([P, 512], F32, tag="pj", bufs=4, name=f"psk{nb}")
                            for nb in range(2)
                        ]
                        for kb in range(EB):
                            for nb in range(2):
                                nc.tensor.matmul(
                                    psk[nb],
                                    lhsT=wk[:, kb, mo * P : (mo + 1) * P],
                                    rhs=xkT[
                                        :, kb, nh * 1024 + nb * 512 : nh * 1024 + (nb + 1) * 512
                                    ],
                                    start=(kb == 0),
                                    stop=(kb == EB - 1),
                                )
                        for nb in range(2):
                            nc.any.tensor_copy(
                                out=kT[
                                    :, mo, nh * 1024 + nb * 512 : nh * 1024 + (nb + 1) * 512
                                ],
                                in_=psk[nb],
                            )

                # V projection -> vones [tok(part), tok-tile, head, 0:64]  (bv deferred)
                for mt in range(KT_TILES):
                    psv = [
                        psC.tile([P, 512], F32, tag="pj", bufs=4, name=f"psv{nb}")
                        for nb in range(EMB // 512)
                    ]
                    for kb in range(EB):
                        for nb in range(EMB // 512):
                            nc.tensor.matmul(
                                psv[nb],
                                lhsT=xkT[:, kb, mt * P : (mt + 1) * P],
                                rhs=wv[:, kb, nb * 512 : (nb + 1) * 512],
                                start=(kb == 0),
                                stop=(kb == EB - 1),
                            )
                    for nb in range(EMB // 512):
                        nc.any.tensor_copy(
                            out=vones[:, mt, 8 * nb : 8 * nb + 8, 0:D],
                            in_=psv[nb].rearrange("p (h d) -> p h d", h=8),
                        )

                if debug_taps:
                    nc.sync.dma_start(dbg["kT"][:], kT[:])
                    nc.sync.dma_start(dbg["vones"][:], vones[:])

            # ---------------- phase D: attention + output projection ------------------
            with (
                tc.tile_pool(name="phD", bufs=1) as pd,
                tc.tile_pool(name="psumE", bufs=1, space="PSUM") as psE,
                tc.tile_pool(name="psumPV", bufs=1, space="PSUM") as psPV,
                tc.tile_pool(name="psumMisc", bufs=1, space="PSUM") as psM,
            ):
                att = pd.tile([P, EB, NQ], F16, tag="att")
                s_all = pd.tile([H, NQ], F32, tag="s_all")
                srec = pd.tile([H, NQ], F16, tag="srec")
                # E[s, t, p] = 1 iff s == 2t + (p >= 64): broadcast selector so a
                # [16, q] vector of per-head values scatters to [128, q] tiles.
                emat = pd.tile([H, EB, P], F16, tag="emat")
                nc.vector.memset(emat, 0.0)
                nc.gpsimd.affine_select(
                    out=emat[:, :, 0:D],
                    in_=emat[:, :, 0:D],
                    # iota = s - 2t over (partition s, free (t, p<64))
                    pattern=[[-2, EB], [0, D]],
                    channel_multiplier=1,
                    base=0,
                    compare_op=mybir.AluOpType.not_equal,
                    fill=1.0,
                )
                nc.gpsimd.affine_select(
                    out=emat[:, :, D:P],
                    in_=emat[:, :, D:P],
                    # iota = s - 2t - 1 over (partition s, free (t, p>=64))
                    pattern=[[-2, EB], [0, D]],
                    channel_multiplier=1,
                    base=-1,
                    compare_op=mybir.AluOpType.not_equal,
                    fill=1.0,
                )

                for qh in range(NQ // 512):
                    qs = slice(qh * 512, (qh + 1) * 512)
                    for hp in range(H // 2):
                        eb = hp
                        pv_ps = [
                            psPV.tile(
                                [D + 1, 512], F32, tag="pv", bufs=2, name=f"pv{s}"
                            )
                            for s in range(2)
                        ]
                        for jh in range(2):
                            ex = pd.tile([P, 2, 8, 512], F16, tag="expT", bufs=3)
                            for j in range(8):
                                kt = jh * 8 + j
                                pe = psE.tile([P, 2, 512], F32, tag="eT", bufs=2)
                                for s in range(2):
                                    r = slice(64 * s, 64 * s + 64)
                                    nc.tensor.matmul(
                                        pe[:, s, :],
                                        lhsT=kT[r, eb, kt * P : (kt + 1) * P],
                                        rhs=qT[r, eb, qs],
                                        start=True,
                                        stop=True,
                                    )
                                nc.scalar.activation(
                                    ex[:, :, j, :],
                                    pe,
                                    mybir.ActivationFunctionType.Exp,
                                    bias=0.0,
                                    scale=SCALE,
                                )
                                for s in range(2):
                                    nc.tensor.matmul(
                                        pv_ps[s],
                                        lhsT=vones[:, kt, 2 * hp + s, :],
                                        rhs=ex[:, s, j, :],
                                        start=(kt == 0),
                                        stop=(kt == KT_TILES - 1),
                                    )
                        # evacuate PV: out rows -> att (head s at partitions 64s..),
                        # denominator row 64 -> s_all via staging + SBUF-to-SBUF DMA
                        for s in range(2):
                            h = 2 * hp + s
                            nc.vector.tensor_copy(
                                out=att[64 * s : 64 * s + 64, eb, qs],
                                in_=pv_ps[s][0:D, :],
                            )
                            sst = pd.tile([65, 512], F32, tag="sstage", bufs=2)
                            nc.vector.tensor_copy(
                                out=sst[64:65, :], in_=pv_ps[s][D : D + 1, :]
                            )
                            nc.sync.dma_start(s_all[h : h + 1, qs], sst[64:65, :])

                    # normalization: srec = 1/s_all ; broadcast with E-matmul; * att; + bv
                    with nc.allow_low_precision(
                        reason="1/S in fp16: one multiplicative factor, ~5e-4 rel"
                    ):
                        nc.vector.reciprocal(srec[:, qs], s_all[:, qs])
                    for t in range(EB):
                        psb = psM.tile([P, 512], F32, tag="misc", bufs=2, name="srecB")
                        nc.tensor.matmul(
                            psb,
                            lhsT=emat[:, t, :],
                            rhs=srec[:, qs],
                            start=True,
                            stop=True,
                        )
                        # NOTE: bv is NOT added here — softmax rows sum to 1, so
                        # bv's contribution to y is exactly bv @ Wp, folded into bpp.
                        nc.vector.tensor_tensor(
                            att[:, t, qs], att[:, t, qs], psb, mybir.AluOpType.mult
                        )

                    # output projection for this q-half; nb-inner reuses loaded att
                    # weights, two PSUM banks ping-pong from the shared misc pool
                    for qt in range(4):
                        rows = slice(qh * 512 + qt * P, qh * 512 + (qt + 1) * P)
                        py = [
                            psM.tile([P, 512], F32, tag="misc", bufs=2, name=f"py{nb}")
                            for nb in range(EMB // 512)
                        ]
                        for kb in range(EB):
                            for nb in range(EMB // 512):
                                nc.tensor.matmul(
                                    py[nb],
                                    lhsT=att[:, kb, rows],
                                    rhs=wp[:, kb, nb * 512 : (nb + 1) * 512],
                                    start=(kb == 0),
                                    stop=False,
                                )
                        for nb in range(EMB // 512):
                            nc.tensor.matmul(
                                py[nb],
                                lhsT=ones_row,
                                rhs=bpp[0:1, nb * 512 : (nb + 1) * 512],
                                start=False,
                                stop=True,
                            )
                            ysb = pd.tile([P, 512], F32, tag="ysb", bufs=2)
                            nc.vector.tensor_copy(out=ysb, in_=py[nb])
                            nc.sync.dma_start(
                                y[rows, nb * 512 : (nb + 1) * 512], ysb
                            )
                if debug_taps:
                    nc.sync.dma_start(dbg["emat"][:], emat[:])
                    nc.sync.dma_start(dbg["s_all"][:], s_all[:])
                    nc.sync.dma_start(dbg["att"][:], att[:])
    return nc


_CACHED = None


def build():
    global _CACHED
    if _CACHED is None:
        nc = bacc.Bacc("TRN2", target_bir_lowering=False, debug=False)
        build_ir(nc)
        nc.compile()
        _CACHED = nc
    return _CACHED


INPUT_NAMES = [
    "query_tokens",
    "image_embeds",
    "Wq",
    "Wk",
    "Wv",
    "Wp",
    "bq",
    "bv",
    "bp",
]


def make_in_maps(inputs):
    arrs = {k: np.ascontiguousarray(np.asarray(v, dtype=np.float32)) for k, v in inputs.items()}
    in_maps = []
    for b in range(NCORES):
        m = {
            "query_tokens": np.ascontiguousarray(arrs["query_tokens"][b]),
            "image_embeds": np.ascontiguousarray(arrs["image_embeds"][b]),
        }
        for n in ("Wq", "Wk", "Wv", "Wp", "bq", "bv", "bp"):
            m[n] = arrs[n]
        in_maps.append(m)
    return in_maps


def run(inputs, trace=False, **kwargs):
    from concourse.bass_utils import run_bass_kernel_spmd

    nc = build()
    res = run_bass_kernel_spmd(
        nc, make_in_maps(inputs), core_ids=list(range(NCORES)), trace=trace, **kwargs
    )
    out = np.stack([r["y"] for r in res.results], axis=0)
    return out, res


def kernel(**inputs) -> np.ndarray:
    out, _ = run(inputs, trace=False)
    return out


# revision 18
# speedup vs baseline: 1.1440x; 1.1163x over previous
# MultiHeadCrossAttention Trainium2 Bass/Tile kernel.
#
# Problem: B=8, NQ=1024, NK=2048, EMB=1024, H=16, D=64 (fp32 I/O).
#   q = query_tokens @ Wq + bq ; k = image_embeds @ Wk + bk ; v = image_embeds @ Wv + bv
#   att = softmax(q k^T / sqrt(EMB)) ; out = (att v) @ Wp + bp
#
# Sharding: data-parallel over batch — core b computes batch element b. No collectives.
#
# Per-core dataflow (all layouts chosen so TensorE contraction is always on partitions):
#   xqT/xkT  = transposed inputs  [emb_in(part-tiles), tokens] fp16 (PE transpose)
#   qT,kT    = Wq/Wk proj outputs [emb(part), tokens] fp16     (bk dropped: softmax-invariant)
#   vones    = V proj [tok(part), head, 64+1] fp16, col 64 = 1.0 (ones col makes PV also
#              produce the softmax denominator row; bv folded into the out-proj bias)
#   eT       = K_h^T.T @ Q_h^T -> PSUM [k-tok(part), q]        (per head, per 128-tok tile)
#   expT     = exp(eT/32) fp16 (ScalarE, scale fused; logits are ~N(0,0.083) so no
#              max-subtraction is needed for stability)
#   O_unnorm = vones.T @ expT -> PSUM [65, q]: rows 0-63 = head out^T, row 64 = sum_k exp
#   att      = O/S gathered to [emb(part), q] fp16, normalized via 1/S broadcast from a
#              tiny PE matmul (E-matrix selector)
#   y        = att.T @ Wp + ones x (bv@Wp + bp) -> [q(part), emb] fp32 -> DRAM
import numpy as np

import concourse.bass as bass
import concourse.mybir as mybir
import concourse.tile as tile
from concourse import bacc
from concourse.masks import make_identity

F32 = mybir.dt.float32
F16 = mybir.dt.float16

B, NQ, NK = 8, 1024, 2048
EMB = 1024
H = 16
D = 64
P = 128
NCORES = 8

QT_TILES = NQ // P        # 8 q-token tiles
KT_TILES = NK // P        # 16 k-token tiles
EB = EMB // P             # 8 emb blocks
SCALE = 1.0 / float(np.sqrt(EMB))


def build_ir(nc, debug_taps=False):
    xq = nc.dram_tensor("query_tokens", [NQ, EMB], F32, kind="ExternalInput")
    xkv = nc.dram_tensor("image_embeds", [NK, EMB], F32, kind="ExternalInput")
    wq_d = nc.dram_tensor("Wq", [EMB, EMB], F32, kind="ExternalInput")
    wk_d = nc.dram_tensor("Wk", [EMB, EMB], F32, kind="ExternalInput")
    wv_d = nc.dram_tensor("Wv", [EMB, EMB], F32, kind="ExternalInput")
    wp_d = nc.dram_tensor("Wp", [EMB, EMB], F32, kind="ExternalInput")
    bq_d = nc.dram_tensor("bq", [EMB], F32, kind="ExternalInput")
    bv_d = nc.dram_tensor("bv", [EMB], F32, kind="ExternalInput")
    bp_d = nc.dram_tensor("bp", [EMB], F32, kind="ExternalInput")
    y = nc.dram_tensor("y", [NQ, EMB], F32, kind="ExternalOutput")
    dbg = {}
    if debug_taps:
        dbg["qT"] = nc.dram_tensor("dbg_qT", [P, EB, NQ], F16, kind="ExternalOutput")
        dbg["kT"] = nc.dram_tensor("dbg_kT", [P, EB, NK], F16, kind="ExternalOutput")
        dbg["vones"] = nc.dram_tensor(
            "dbg_vones", [P, KT_TILES, H, D + 1], F16, kind="ExternalOutput"
        )
        dbg["emat"] = nc.dram_tensor("dbg_emat", [H, EB, P], F16, kind="ExternalOutput")
        dbg["s_all"] = nc.dram_tensor("dbg_s_all", [H, NQ], F32, kind="ExternalOutput")
        dbg["att"] = nc.dram_tensor("dbg_att", [P, EB, NQ], F16, kind="ExternalOutput")
        dbg["xqT"] = nc.dram_tensor("dbg_xqT", [P, EB, NQ], F16, kind="ExternalOutput")
        dbg["bpp"] = nc.dram_tensor("dbg_bpp", [1, EMB], F16, kind="ExternalOutput")

    with tile.TileContext(nc) as tc:
        with tc.tile_pool(name="persist", bufs=1) as pp:
            ident = pp.tile([P, P], F16, tag="ident")
            make_identity(nc, ident)
            bq_sb = pp.tile([P, EB], F32, tag="bq")
            bv_sb = pp.tile([P, EB], F32, tag="bv")
            with nc.allow_non_contiguous_dma(reason="tiny bias loads"):
                nc.sync.dma_start(bq_sb, bq_d[:].rearrange("(b p) -> p b", p=P))
                nc.sync.dma_start(bv_sb, bv_d[:].rearrange("(b p) -> p b", p=P))
            ones_row = pp.tile([1, P], F16, tag="ones_row")
            nc.vector.memset(ones_row, 1.0)
            bv16 = pp.tile([P, EB], F16, tag="bv16")
            nc.vector.tensor_copy(out=bv16, in_=bv_sb)

            qT = pp.tile([P, EB, NQ], F16, tag="qT")
            kT = pp.tile([P, EB, NK], F16, tag="kT")
            vones = pp.tile([P, KT_TILES, H, D + 1], F16, tag="vones")
            nc.vector.memset(vones[:, :, :, D : D + 1], 1.0)

            # ------------- phase A-C: load/cast, transpose, project Q, K, V ----------
            with (
                tc.tile_pool(name="abc", bufs=1) as pa,
                tc.tile_pool(name="psumABC", bufs=1, space="PSUM") as psA,
            ):
                wq = pa.tile([P, EB, EMB], F16, tag="wq")
                wk = pa.tile([P, EB, EMB], F16, tag="wk")
                wv = pa.tile([P, EB, EMB], F16, tag="wv")
                xqT = pa.tile([P, EB, NQ], F16, tag="xqT")
                xkT = pa.tile([P, EB, NK], F16, tag="xkT")

                def transpose_in(dst, src_dram, tt):
                    xnat = pa.tile([P, EMB], F32, tag="xnat", bufs=2)
                    nc.sync.dma_start(xnat, src_dram[tt * P : (tt + 1) * P, :])
                    xnat16 = pa.tile([P, EMB], F16, tag="xnat16", bufs=2)
                    nc.vector.tensor_copy(out=xnat16, in_=xnat)
                    for g in range(2):
                        tp = psA.tile([P, 4 * P], F16, tag="tp", bufs=3)
                        for e4 in range(4):
                            eb = 4 * g + e4
                            nc.tensor.transpose(
                                tp[:, e4 * P : (e4 + 1) * P],
                                xnat16[:, eb * P : (eb + 1) * P],
                                ident,
                            )
                        nc.vector.tensor_copy(
                            out=dst[:, 4 * g : 4 * g + 4, tt * P : (tt + 1) * P],
                            in_=tp.rearrange("p (b f) -> p b f", b=4),
                        )

                def load_w(w16, wd):
                    for kb in range(EB):
                        wstage = pa.tile([P, EMB], F32, tag="wstage", bufs=4)
                        nc.sync.dma_start(wstage, wd[kb * P : (kb + 1) * P, :])
                        nc.any.tensor_copy(out=w16[:, kb, :], in_=wstage)

                # inputs first (PE transposes are the first compute), weights behind
                for tt in range(QT_TILES):
                    transpose_in(xqT, xq, tt)
                load_w(wq, wq_d)
                for tt in range(KT_TILES):
                    transpose_in(xkT, xkv, tt)
                load_w(wk, wk_d)
                load_w(wv, wv_d)

                # Q projection: qT[emb, q] = Wq.T-contraction, + bq, cast fp16.
                # nb-inner so consecutive matmuls reuse the loaded weights.
                for mo in range(EB):
                    psq = [
                        psA.tile([P, 512], F32, tag="pj", bufs=4, name=f"psq{nb}")
                        for nb in range(NQ // 512)
                    ]
                    for kb in range(EB):
                        for nb in range(NQ // 512):
                            nc.tensor.matmul(
                                psq[nb],
                                lhsT=wq[:, kb, mo * P : (mo + 1) * P],
                                rhs=xqT[:, kb, nb * 512 : (nb + 1) * 512],
                                start=(kb == 0),
                                stop=(kb == EB - 1),
                            )
                    for nb in range(NQ // 512):
                        nc.any.tensor_scalar_add(
                            qT[:, mo, nb * 512 : (nb + 1) * 512],
                            psq[nb],
                            bq_sb[:, mo : mo + 1],
                        )

                # K projection (no bias: bk is softmax-invariant)
                for mo in range(EB):
                    for nh in range(NK // 1024):
                        psk = [
                            psA.tile([P, 512], F32, tag="pj", bufs=4, name=f"psk{nb}")
                            for nb in range(2)
                        ]
                        for kb in range(EB):
                            for nb in range(2):
                                nc.tensor.matmul(
                                    psk[nb],
                                    lhsT=wk[:, kb, mo * P : (mo + 1) * P],
                                    rhs=xkT[
                                        :, kb,
                                        nh * 1024 + nb * 512 : nh * 1024 + (nb + 1) * 512,
                                    ],
                                    start=(kb == 0),
                                    stop=(kb == EB - 1),
                                )
                        for nb in range(2):
                            nc.any.tensor_copy(
                                out=kT[
                                    :, mo,
                                    nh * 1024 + nb * 512 : nh * 1024 + (nb + 1) * 512,
                                ],
                                in_=psk[nb],
                            )

                # V projection -> vones [tok(part), tok-tile, head, 0:64]  (bv deferred)
                for mt in range(KT_TILES):
                    psv = [
                        psA.tile([P, 512], F32, tag="pj", bufs=4, name=f"psv{nb}")
                        for nb in range(EMB // 512)
                    ]
                    for kb in range(EB):
                        for nb in range(EMB // 512):
                            nc.tensor.matmul(
                                psv[nb],
                                lhsT=xkT[:, kb, mt * P : (mt + 1) * P],
                                rhs=wv[:, kb, nb * 512 : (nb + 1) * 512],
                                start=(kb == 0),
                                stop=(kb == EB - 1),
                            )
                    for nb in range(EMB // 512):
                        nc.any.tensor_copy(
                            out=vones[:, mt, 8 * nb : 8 * nb + 8, 0:D],
                            in_=psv[nb].rearrange("p (h d) -> p h d", h=8),
                        )

                if debug_taps:
                    nc.sync.dma_start(dbg["xqT"][:], xqT[:])
                    nc.sync.dma_start(dbg["qT"][:], qT[:])
                    nc.sync.dma_start(dbg["kT"][:], kT[:])
                    nc.sync.dma_start(dbg["vones"][:], vones[:])

            # ---------------- phase D: attention + output projection ------------------
            with (
                tc.tile_pool(name="phD", bufs=1) as pd,
                tc.tile_pool(name="psumE", bufs=1, space="PSUM") as psE,
                tc.tile_pool(name="psumPV", bufs=1, space="PSUM") as psPV,
                tc.tile_pool(name="psumMisc", bufs=1, space="PSUM") as psM,
            ):
                att = pd.tile([P, EB, NQ], F16, tag="att")
                s_all = pd.tile([H, NQ], F32, tag="s_all")
                srec32 = pd.tile([H, NQ], F32, tag="srec32")
                srec = pd.tile([H, NQ], F16, tag="srec")
                wp = pd.tile([P, EB, EMB], F16, tag="wp")
                bpp = pd.tile([1, EMB], F16, tag="bpp")  # bv @ Wp + bp
                bp_sb = pd.tile([1, EMB], F32, tag="bp_sb")
                nc.sync.dma_start(bp_sb, bp_d[None, :])
                for kb in range(EB):
                    wstage = pd.tile([P, EMB], F32, tag="wstage", bufs=2)
                    nc.sync.dma_start(wstage, wp_d[kb * P : (kb + 1) * P, :])
                    nc.any.tensor_copy(out=wp[:, kb, :], in_=wstage)

                # bpp = bv @ Wp + bp  (rank-1 bias prep for the output projection)
                for nb in range(EMB // 512):
                    psb = psM.tile([P, 512], F32, tag="misc", bufs=2, name="bp_ps")
                    for kb in range(EB):
                        nc.tensor.matmul(
                            psb[0:1, :],
                            lhsT=bv16[:, kb : kb + 1],
                            rhs=wp[:, kb, nb * 512 : (nb + 1) * 512],
                            start=(kb == 0),
                            stop=(kb == EB - 1),
                        )
                    nc.vector.tensor_tensor(
                        bpp[0:1, nb * 512 : (nb + 1) * 512],
                        psb[0:1, :],
                        bp_sb[0:1, nb * 512 : (nb + 1) * 512],
                        mybir.AluOpType.add,
                    )

                # E[s, t, p] = 1 iff s == 2t + (p >= 64): broadcast selector so a
                # [16, q] vector of per-head values scatters to [128, q] tiles.
                emat = pd.tile([H, EB, P], F16, tag="emat")
                nc.vector.memset(emat, 0.0)
                nc.gpsimd.affine_select(
                    out=emat[:, :, 0:D],
                    in_=emat[:, :, 0:D],
                    pattern=[[-2, EB], [0, D]],
                    channel_multiplier=1,
                    base=0,
                    compare_op=mybir.AluOpType.not_equal,
                    fill=1.0,
                )
                nc.gpsimd.affine_select(
                    out=emat[:, :, D:P],
                    in_=emat[:, :, D:P],
                    pattern=[[-2, EB], [0, D]],
                    channel_multiplier=1,
                    base=-1,
                    compare_op=mybir.AluOpType.not_equal,
                    fill=1.0,
                )

                for qh in range(NQ // 512):
                    qs = slice(qh * 512, (qh + 1) * 512)
                    for hp in range(H // 2):
                        eb = hp
                        pv_ps = [
                            psPV.tile(
                                [D + 1, 512], F32, tag="pv", bufs=2, name=f"pv{s}"
                            )
                            for s in range(2)
                        ]
                        for jh in range(2):
                            ex = pd.tile([P, 2, 8, 512], F16, tag="expT", bufs=3)
                            for j in range(8):
                                kt = jh * 8 + j
                                pe = psE.tile([P, 2, 512], F32, tag="eT", bufs=2)
                                for s in range(2):
                                    r = slice(64 * s, 64 * s + 64)
                                    nc.tensor.matmul(
                                        pe[:, s, :],
                                        lhsT=kT[r, eb, kt * P : (kt + 1) * P],
                                        rhs=qT[r, eb, qs],
                                        start=True,
                                        stop=True,
                                    )
                                nc.scalar.activation(
                                    ex[:, :, j, :],
                                    pe,
                                    mybir.ActivationFunctionType.Exp,
                                    bias=0.0,
                                    scale=SCALE,
                                )
                                for s in range(2):
                                    nc.tensor.matmul(
                                        pv_ps[s],
                                        lhsT=vones[:, kt, 2 * hp + s, :],
                                        rhs=ex[:, s, j, :],
                                        start=(kt == 0),
                                        stop=(kt == KT_TILES - 1),
                                    )
                        # evacuate PV: out rows -> att (head s at partitions 64s..),
                        # denominator row 64 -> s_all via staging + SBUF-to-SBUF DMA
                        for s in range(2):
                            h = 2 * hp + s
                            nc.vector.tensor_copy(
                                out=att[64 * s : 64 * s + 64, eb, qs],
                                in_=pv_ps[s][0:D, :],
                            )
                            sst = pd.tile([65, 512], F32, tag="sstage", bufs=2)
                            nc.vector.tensor_copy(
                                out=sst[64:65, :], in_=pv_ps[s][D : D + 1, :]
                            )
                            nc.sync.dma_start(s_all[h : h + 1, qs], sst[64:65, :])

                    # normalization: srec = 1/s_all ; broadcast with E-matmul; * att
                    nc.vector.reciprocal_approx_fast(srec32[:, qs], s_all[:, qs])
                    nc.vector.tensor_copy(out=srec[:, qs], in_=srec32[:, qs])
                    for t in range(EB):
                        psb = psM.tile([P, 512], F32, tag="misc", bufs=2, name="srecB")
                        nc.tensor.matmul(
                            psb,
                            lhsT=emat[:, t, :],
                            rhs=srec[:, qs],
                            start=True,
                            stop=True,
                        )
                        # bv is NOT added here — softmax rows sum to 1, so bv's
                        # contribution to y is exactly bv @ Wp, folded into bpp.
                        nc.vector.tensor_tensor(
                            att[:, t, qs], att[:, t, qs], psb, mybir.AluOpType.mult
                        )

                    # output projection for this q-half; nb-inner reuses loaded att
                    # weights, two PSUM banks ping-pong from the shared misc pool
                    for qt in range(4):
                        rows = slice(qh * 512 + qt * P, qh * 512 + (qt + 1) * P)
                        py = [
                            psM.tile([P, 512], F32, tag="misc", bufs=2, name=f"py{nb}")
                            for nb in range(EMB // 512)
                        ]
                        for kb in range(EB):
                            for nb in range(EMB // 512):
                                nc.tensor.matmul(
                                    py[nb],
                                    lhsT=att[:, kb, rows],
                                    rhs=wp[:, kb, nb * 512 : (nb + 1) * 512],
                                    start=(kb == 0),
                                    stop=False,
                                )
                        for nb in range(EMB // 512):
                            nc.tensor.matmul(
                                py[nb],
                                lhsT=ones_row,
                                rhs=bpp[0:1, nb * 512 : (nb + 1) * 512],
                                start=False,
                                stop=True,
                            )
                            ysb = pd.tile([P, 512], F32, tag="ysb", bufs=2)
                            nc.vector.tensor_copy(out=ysb, in_=py[nb])
                            nc.sync.dma_start(
                                y[rows, nb * 512 : (nb + 1) * 512], ysb
                            )
                if debug_taps:
                    nc.sync.dma_start(dbg["emat"][:], emat[:])
                    nc.sync.dma_start(dbg["s_all"][:], s_all[:])
                    nc.sync.dma_start(dbg["att"][:], att[:])
                    nc.sync.dma_start(dbg["bpp"][:], bpp[:])
    return nc


_CACHED = None


def build():
    global _CACHED
    if _CACHED is None:
        nc = bacc.Bacc("TRN2", target_bir_lowering=False, debug=False)
        build_ir(nc)
        nc.compile()
        _CACHED = nc
    return _CACHED


INPUT_NAMES = [
    "query_tokens",
    "image_embeds",
    "Wq",
    "Wk",
    "Wv",
    "Wp",
    "bq",
    "bv",
    "bp",
]


def make_in_maps(inputs):
    arrs = {k: np.ascontiguousarray(np.asarray(v, dtype=np.float32)) for k, v in inputs.items()}
    in_maps = []
    for b in range(NCORES):
        m = {
            "query_tokens": np.ascontiguousarray(arrs["query_tokens"][b]),
            "image_embeds": np.ascontiguousarray(arrs["image_embeds"][b]),
        }
        for n in ("Wq", "Wk", "Wv", "Wp", "bq", "bv", "bp"):
            m[n] = arrs[n]
        in_maps.append(m)
    return in_maps


def run(inputs, trace=False, **kwargs):
    from concourse.bass_utils import run_bass_kernel_spmd

    nc = build()
    res = run_bass_kernel_spmd(
        nc, make_in_maps(inputs), core_ids=list(range(NCORES)), trace=trace, **kwargs
    )
    out = np.stack([r["y"] for r in res.results], axis=0)
    return out, res


def kernel(**inputs) -> np.ndarray:
    out, _ = run(inputs, trace=False)
    return out
